# revision 20
# baseline (speedup 1.0000x reference)
"""CMAttention Trainium2 Bass kernel.

Reference computation (b=2, n=2048, dim=512, H=8 heads, dh=64, M=3 memory tokens):
    q = x @ wq;  k, v = split(x @ wkv);  per-head attention with 3 extra
    memory k/v tokens appended;  out = softmax(q k^T / 8) v;  y = out @ wo + bo.

Sharding: 16 (batch, head) pairs over 8 cores -> each core owns one batch and
two adjacent heads.  Per core everything is local; the out-projection is
row-sharded (per-head) and partial outputs are summed on the host (the
all-reduce of the sharding hint, done at gather time).

Device-side layout (per core, two heads "stacked" on partitions 0-63 / 64-127):
    xt   [4][128, 2048]   x[b]^T in bf16, contraction c on partitions
    qT   [128, 2048]      q^T = wq_s^T-chunks @ xt    (d_global on partitions)
    kT   [128, 2052]      k^T * 1/8 (scale folded into wk on host) ++ memory keys
    v    via PE transpose -> v_aug[h] [128, 17*65]: per j-tile [128, 64+1(ones)]
    scores^T s[j, i] = kT_h^T-slice.T @ qT_h  -> PSUM [128(j), 1024(i)]
      (the two heads' QK matmuls dual-issue on the PE's 64-row groups)
    exp on ScalarE PSUM->SBUF
    av:  out_h^T[65, i] += v_aug_jt.T @ exp_jt   (row 64 = softmax denominator)
    denominator row -> rec_col via tiny PE transposes (no DRAM round-trip)
    out-projection per head + per-partition reciprocal scaling, host sums partials.
"""

import sys

for _p in ("/opt/trn_rl_repo", "/root/.axon_site/_ro/trn_rl_repo"):
    if _p not in sys.path:
        sys.path.insert(0, _p)

import ml_dtypes
import numpy as np

import concourse.bacc as bacc
import concourse.mybir as mybir
import concourse.tile as tile
from concourse import bass_utils
from concourse.masks import make_identity

F32 = mybir.dt.float32
BF = mybir.dt.bfloat16
AF = mybir.ActivationFunctionType
ALU = mybir.AluOpType

H, DH, M = 8, 64, 3
DIM = 512
INNER = H * DH
NSEQ = 2048
B = 2
N_CORES = 8
SCALE = DH ** -0.5
SQRT_M = float(np.sqrt(M))

_CACHE = {}


def _emit(nc, tc, n):
    """Emit the per-core program. n = sequence length (queries)."""
    n_it = n // 128          # i-tiles of 128 queries
    n_jt = n // 128 + 1      # j-tiles: n/128 full + 1 memory tile (3 rows)
    VA = 65                  # v_aug cols per j-tile: 64 dims + ones column

    ap_xt = nc.dram_tensor("xt", [4, 128, n], BF, kind="ExternalInput").ap()
    # weights stored chunk-major along columns: [128, 4*128], col block c =
    # contraction chunk c -> one contiguous 1KB-row DMA instead of 4 small ones
    ap_wq = nc.dram_tensor("wq_s", [128, 512], BF, kind="ExternalInput").ap()
    ap_wk = nc.dram_tensor("wk_s", [128, 512], BF, kind="ExternalInput").ap()
    ap_wv = nc.dram_tensor("wv_s", [128, 512], BF, kind="ExternalInput").ap()
    ap_wo = nc.dram_tensor("wo_s", [128, DIM], BF, kind="ExternalInput").ap()
    ap_mkT = nc.dram_tensor("mkT_s", [128, M], BF, kind="ExternalInput").ap()
    ap_mv = nc.dram_tensor("mv_s", [M, 128], BF, kind="ExternalInput").ap()
    ap_out = nc.dram_tensor("out", [n_it, 128, DIM], BF, kind="ExternalOutput").ap()

    with (
        tc.tile_pool(name="persist", bufs=1) as per,
    ):
        xt = [per.tile([128, n], BF, tag=f"xt{c}", name=f"xt{c}") for c in range(4)]
        wq_all = per.tile([128, 512], BF, tag="wq", name="wq")
        wk_all = per.tile([128, 512], BF, tag="wk", name="wk")
        wv_all = per.tile([128, 512], BF, tag="wv", name="wv")
        wq_sb = [wq_all[:, c * 128 : (c + 1) * 128] for c in range(4)]
        wk_sb = [wk_all[:, c * 128 : (c + 1) * 128] for c in range(4)]
        wv_sb = [wv_all[:, c * 128 : (c + 1) * 128] for c in range(4)]
        wo_sb = per.tile([128, DIM], BF, tag="wo", name="wo")
        qT = per.tile([128, n], BF, tag="qT", name="qT")
        kT = per.tile([128, n + 128], BF, tag="kT", name="kT")
        vT = per.tile([128, n], BF, tag="vT", name="vT")
        v_aug = [per.tile([128, n_jt * VA], BF, tag=f"vaug{h}", name=f"vaug{h}") for h in range(2)]
        oT = per.tile([128, n], BF, tag="oT", name="oT")
        rec_col = per.tile([128, 2 * n_it], F32, tag="rec", name="rec")
        ident = per.tile([128, 128], BF, tag="ident", name="ident")
        ones1 = per.tile([1, 1], F32, tag="ones1", name="ones1")

        # ---- input DMAs: 3 hardware queues (sync/scalar/gpsimd) at ~77B/ns
        # each.  xt split into 16 [128,512] pieces issued round-robin in the
        # k-projection's consumption order (k-chunk outer, c inner) so the
        # PE can start on kT chunk 0 ~2 pieces in.  wk first (needed first),
        # wq early on gpsimd (needed by q0 right after kT chunk 0).
        piece = []
        for k in range(4):
            for c in range(4):
                piece.append(
                    (xt[c][:, k * 512 : (k + 1) * 512],
                     ap_xt[c][:, k * 512 : (k + 1) * 512])
                )
        rest = piece[4:]
        sc = [piece[0], piece[2]] + rest[0::3]
        gp = [(wq_all, ap_wq), piece[1]] + rest[1::3]
        sy = [(wk_all, ap_wk), piece[3]] + rest[2::3]
        for i in range(max(len(sc), len(gp), len(sy))):
            if i < len(sc):
                nc.scalar.dma_start(out=sc[i][0], in_=sc[i][1])
            if i < len(gp):
                nc.gpsimd.dma_start(out=gp[i][0], in_=gp[i][1])
            if i < len(sy):
                nc.sync.dma_start(out=sy[i][0], in_=sy[i][1])
        nc.sync.dma_start(out=wv_all, in_=ap_wv)
        nc.sync.dma_start(out=wo_sb, in_=ap_wo)
        nc.vector.memset(kT[:, n : n + 128], 0.0)
        nc.sync.dma_start(out=kT[0:64, n : n + M], in_=ap_mkT[0:64, :])
        nc.sync.dma_start(out=kT[64:128, n + 64 : n + 64 + M], in_=ap_mkT[64:128, :])
        make_identity(nc, ident[:])
        nc.gpsimd.memset(ones1[:], 1.0)
        for h in range(2):
            hm = h * 64  # mem rows: h0 at 0:3, h1 at 64:67 (dual-issue pair)
            nc.vector.memset(v_aug[h][:], 1.0)
            mb = (n_jt - 1) * VA
            nc.vector.memset(v_aug[h][:, mb : mb + VA], 0.0)
            nc.vector.memset(v_aug[h][hm : hm + M, mb + 64 : mb + VA], 1.0)
            nc.sync.dma_start(
                out=v_aug[h][hm : hm + M, mb : mb + 64],
                in_=ap_mv[:, h * 64 : (h + 1) * 64],
            )

        # ---- minimal pre-attention projections: kT chunk 0 and qT chunk 0
        # only — attention starts as soon as they land; kT chunks 1-3, vT,
        # transposes and remaining qT chunks ride inside the Q0 window as
        # PE filler while the rest of x streams in.
        with tc.tile_pool(name="proj_ps", bufs=4, space="PSUM") as proj_ps:
            for name, w_sb, dst in (("kps0", wk_sb, kT), ("q0ps", wq_sb, qT)):
                ps = proj_ps.tile([128, 512], F32, tag="proj", name=name)
                for c in range(4):
                    nc.tensor.matmul(
                        ps[:],
                        w_sb[c][:],
                        xt[c][:, 0:512],
                        start=(c == 0),
                        stop=(c == 3),
                    )
                nc.scalar.copy(out=dst[:, 0:512], in_=ps[:])

        # ---- attention: i-quarter (512) outer; both heads share one scores
        # PSUM tile (h0 cols 0-511, h1 cols 512-1023) -> one exp call covers
        # both heads; sp double-buffered; av staggered one j-tile behind.
        # The out-projection for quarter q-1 rides inside quarter q's window;
        # its PSUM tiles share the "mix" pool with the av accumulators and
        # deferred q/v projections.
        n_iq = n // 512

        def outproj_quarter(iq, half, mix_pool, stage_pool, tail=False, sp_pool=None):
            ts0 = iq * 4 + (2 if half else 0)
            for t in range(ts0, ts0 + 2):
                if tail:
                    p01 = sp_pool.tile([128, 1024], F32, tag="sp", name="p01")
                    p0 = p01[:, 0:512]
                    p1 = p01[:, 512:1024]
                else:
                    p0 = mix_pool.tile([128, 512], F32, tag="mix", name="p0")
                    p1 = mix_pool.tile([128, 512], F32, tag="mix", name="p1")
                nc.tensor.matmul(
                    p0[:],
                    oT[0:64, t * 128 : (t + 1) * 128],
                    wo_sb[0:64, :],
                    start=True,
                    stop=True,
                )
                nc.tensor.matmul(
                    p1[:],
                    oT[64:128, t * 128 : (t + 1) * 128],
                    wo_sb[64:128, :],
                    start=True,
                    stop=True,
                )
                a1 = stage_pool.tile([128, 512], F32, tag="a1", name="a1")
                if tail:
                    # ACT is idle after the last exp — do the h1 scale there so
                    # the DVE only runs one op per tile on the critical tail.
                    nc.scalar.activation(
                        out=a1[:],
                        in_=p1[:],
                        func=AF.Copy,
                        scale=rec_col[:, n_it + t : n_it + t + 1],
                    )
                else:
                    nc.vector.tensor_scalar_mul(
                        a1[:], p1[:], rec_col[:, n_it + t : n_it + t + 1]
                    )
                outb = stage_pool.tile([128, 512], BF, tag="outb", name="outb")
                nc.vector.scalar_tensor_tensor(
                    out=outb[:],
                    in0=p0[:],
                    scalar=rec_col[:, t : t + 1],
                    in1=a1[:],
                    op0=ALU.mult,
                    op1=ALU.add,
                )
                if tail:
                    eng = (nc.sync, nc.gpsimd, nc.scalar)[t % 3]
                else:
                    eng = nc.sync if t % 2 == 0 else nc.gpsimd
                eng.dma_start(out=ap_out[t], in_=outb[:])

        def kproj_chunk(k, mix_pool):
            kps = mix_pool.tile([128, 512], F32, tag="mix", name=f"kps{k}")
            for c in range(4):
                nc.tensor.matmul(
                    kps[:],
                    wk_sb[c][:],
                    xt[c][:, k * 512 : (k + 1) * 512],
                    start=(c == 0),
                    stop=(c == 3),
                )
            nc.scalar.copy(out=kT[:, k * 512 : (k + 1) * 512], in_=kps[:])

        with (
            tc.tile_pool(name="s_ps", bufs=2, space="PSUM") as s_ps_pool,
            tc.tile_pool(name="mix_ps", bufs=4, space="PSUM") as mix_ps,
            tc.tile_pool(name="exp_sb", bufs=11) as exp_pool,
            tc.tile_pool(name="small", bufs=4) as small,
            tc.tile_pool(name="ostage", bufs=4) as ostage,
        ):
            def do_rec(riq, dens):
                for h in range(2):
                    dc = mix_ps.tile([128, 4], F32, tag="mix", name="dc")
                    for t in range(4):
                        nc.tensor.transpose(
                            dc[:, t : t + 1],
                            dens[h][0:1, t * 128 : (t + 1) * 128],
                            ones1[:],
                        )
                    nc.vector.reciprocal(
                        out=rec_col[:, h * n_it + riq * 4 : h * n_it + riq * 4 + 4],
                        in_=dc[:],
                    )

            deferred_rec = None
            for iq in range(n_iq):
                i0 = iq * 512
                avs = [
                    mix_ps.tile([VA, 512], F32, tag="mix", name=f"av{h}")
                    for h in range(2)
                ]
                pending = []
                for jt in range(n_jt):
                    sp = s_ps_pool.tile([128, 1024], F32, tag="sp", name="sp")
                    for h in range(2):
                        hp = h * 64
                        nc.tensor.matmul(
                            sp[:, h * 512 : (h + 1) * 512],
                            kT[hp : hp + 64, jt * 128 : (jt + 1) * 128],
                            qT[hp : hp + 64, i0 : i0 + 512],
                            start=True,
                            stop=True,
                        )
                    et = exp_pool.tile([128, 1024], BF, tag="exp", name="et")
                    nc.scalar.activation(out=et[:], in_=sp[:], func=AF.Exp)
                    pending.append((et, jt))
                    if jt == 1 and deferred_rec is not None:
                        do_rec(*deferred_rec)
                        deferred_rec = None
                    if iq == 0 and jt in (2, 4, 6):
                        # kT chunks 1-3 ride just behind the arriving x stream
                        kproj_chunk(jt // 2, mix_ps)
                    if iq == 0 and jt in (7, 9):
                        icp = 0 if jt == 7 else 2
                        vps = [
                            mix_ps.tile([128, 512], F32, tag="mix", name="vps")
                            for _ in range(2)
                        ]
                        for c in range(4):
                            for k in range(2):
                                nc.tensor.matmul(
                                    vps[k][:],
                                    wv_sb[c][:],
                                    xt[c][:, (icp + k) * 512 : (icp + k + 1) * 512],
                                    start=(c == 0),
                                    stop=(c == 3),
                                )
                        for k in range(2):
                            nc.vector.tensor_copy(
                                out=vT[:, (icp + k) * 512 : (icp + k + 1) * 512],
                                in_=vps[k][:],
                            )
                    if iq == 0 and jt in (8, 10):
                        t0 = 0 if jt == 8 else 8
                        for tjt in range(t0, t0 + 8):
                            pt = mix_ps.tile([128, 128], BF, tag="mix", name="tr")
                            nc.tensor.transpose(
                                pt[:], vT[:, tjt * 128 : (tjt + 1) * 128], ident[:]
                            )
                            for h in range(2):
                                nc.vector.tensor_copy(
                                    out=v_aug[h][:, tjt * VA : tjt * VA + 64],
                                    in_=pt[:, h * 64 : (h + 1) * 64],
                                )
                    if iq == 0:
                        n_pop = 0
                        if jt >= 9:
                            n_pop = 2 if len(pending) > 6 else 1
                    else:
                        n_pop = 1 if len(pending) > 2 else 0
                    for _ in range(n_pop):
                        pet, pjt = pending.pop(0)
                        for h in range(2):
                            nc.tensor.matmul(
                                avs[h][:],
                                v_aug[h][:, pjt * VA : (pjt + 1) * VA],
                                pet[:, h * 512 : (h + 1) * 512],
                                start=(pjt == 0),
                                stop=False,
                            )
                    if iq == 0 and jt in (11, 13, 15):
                        # deferred qT chunk, one per insertion point
                        ic = (jt - 11) // 2 + 1
                        qp = mix_ps.tile([128, 512], F32, tag="mix", name="qdef")
                        for c in range(4):
                            nc.tensor.matmul(
                                qp[:],
                                wq_sb[c][:],
                                xt[c][:, ic * 512 : (ic + 1) * 512],
                                start=(c == 0),
                                stop=(c == 3),
                            )
                        nc.vector.tensor_copy(
                            out=qT[:, ic * 512 : (ic + 1) * 512], in_=qp[:]
                        )
                    if jt in (4, 8) and iq >= 1:
                        # out-projection for the previous quarter rides here
                        # (2 tiles per insertion), round-trip long completed
                        outproj_quarter(iq - 1, jt == 8, mix_ps, ostage)
                while pending:
                    pet, pjt = pending.pop(0)
                    for h in range(2):
                        if pjt == n_jt - 1:
                            hm = h * 64
                            nc.tensor.matmul(
                                avs[h][:],
                                v_aug[h][hm : hm + M, pjt * VA : (pjt + 1) * VA],
                                pet[hm : hm + M, h * 512 : (h + 1) * 512],
                                start=False,
                                stop=True,
                            )
                        else:
                            nc.tensor.matmul(
                                avs[h][:],
                                v_aug[h][:, pjt * VA : (pjt + 1) * VA],
                                pet[:, h * 512 : (h + 1) * 512],
                                start=(pjt == 0),
                                stop=False,
                            )
                # epilogue for this i-quarter: pull the denominator rows and
                # oT out of PSUM; the rec computation (PE transposes + recip)
                # is deferred into the next quarter's window so the PE never
                # idles at the boundary.  DVE does the copies mid-run; ACT
                # (idle after the last exp) takes them on the final quarter.
                dens = []
                for h in range(2):
                    den = small.tile([1, 512], F32, tag="den", name="den")
                    if iq == n_iq - 1:
                        nc.scalar.copy(out=den[:], in_=avs[h][64:65, :])
                    else:
                        nc.vector.tensor_copy(out=den[:], in_=avs[h][64:65, :])
                    dens.append(den)
                for h in range(2):
                    hp = h * 64
                    nc.vector.tensor_copy(
                        out=oT[hp : hp + 64, i0 : i0 + 512], in_=avs[h][0:64, :]
                    )
                deferred_rec = (iq, dens)
                if iq == n_iq - 1:
                    do_rec(*deferred_rec)
            # final quarter's out-projection (ACT helps with the scaling)
            outproj_quarter(n_iq - 1, False, mix_ps, ostage, tail=True, sp_pool=s_ps_pool)
            outproj_quarter(n_iq - 1, True, mix_ps, ostage, tail=True, sp_pool=s_ps_pool)


def _build(n=NSEQ):
    if n in _CACHE:
        return _CACHE[n]
    nc = bacc.Bacc("TRN2", debug=False, num_devices=N_CORES)
    with tile.TileContext(nc) as tc:
        _emit(nc, tc, n)
    nc.compile()
    _CACHE[n] = nc
    return nc


def _prep_in_maps(x, wq, wkv, wo, m_k, m_v, n):
    x = np.asarray(x, np.float32)
    wq = np.asarray(wq, np.float32)
    wkv = np.asarray(wkv, np.float32)
    wo = np.asarray(wo, np.float32)
    m_k = np.asarray(m_k, np.float32)
    m_v = np.asarray(m_v, np.float32)

    wk = wkv[:, :INNER]
    wv = wkv[:, INNER:]
    # memory tokens: flat reshape (M, INNER) -> (H, M, DH), exactly as reference
    mk_heads = m_k.reshape(M * INNER).reshape(H, M, DH)  # * SQRT_DH * SCALE == 1.0
    mv_heads = m_v.reshape(M * INNER).reshape(H, M, DH) * SQRT_M

    in_maps = []
    for cid in range(N_CORES):
        b = cid // 4
        h0 = 2 * (cid % 4)
        sl = slice(h0 * DH, (h0 + 2) * DH)
        in_maps.append(
            {
                "xt": np.ascontiguousarray(x[b].T)
                .reshape(4, 128, n)
                .astype(ml_dtypes.bfloat16),
                "wq_s": np.ascontiguousarray(
                    wq[:, sl].reshape(4, 128, 128).transpose(1, 0, 2).reshape(128, 512)
                ).astype(ml_dtypes.bfloat16),
                "wk_s": np.ascontiguousarray(
                    (wk[:, sl] * SCALE)
                    .reshape(4, 128, 128)
                    .transpose(1, 0, 2)
                    .reshape(128, 512)
                ).astype(ml_dtypes.bfloat16),
                "wv_s": np.ascontiguousarray(
                    wv[:, sl].reshape(4, 128, 128).transpose(1, 0, 2).reshape(128, 512)
                ).astype(ml_dtypes.bfloat16),
                "wo_s": np.ascontiguousarray(wo[sl, :]).astype(ml_dtypes.bfloat16),
                "mkT_s": np.ascontiguousarray(
                    np.concatenate([mk_heads[h0].T, mk_heads[h0 + 1].T], axis=0)
                ).astype(ml_dtypes.bfloat16),
                "mv_s": np.ascontiguousarray(
                    np.concatenate([mv_heads[h0], mv_heads[h0 + 1]], axis=1)
                ).astype(ml_dtypes.bfloat16),
            }
        )
    return in_maps


def _gather(results, bo, n):
    bo = np.asarray(bo, np.float32)
    out = np.zeros((B, n, DIM), np.float32)
    for cid in range(N_CORES):
        out[cid // 4] += results[cid]["out"].reshape(n, DIM).astype(np.float32)
    out += bo
    return out


def run(x, wq, wkv, wo, bo, m_k, m_v, trace=False, n=NSEQ):
    nc = _build(n)
    in_maps = _prep_in_maps(x, wq, wkv, wo, m_k, m_v, n)
    res = bass_utils.run_bass_kernel_spmd(
        nc, in_maps, core_ids=list(range(N_CORES)), trace=trace
    )
    return _gather(res.results, bo, n), res


def kernel(x, wq, wkv, wo, bo, m_k, m_v):
    out, _ = run(x, wq, wkv, wo, bo, m_k, m_v)
    return out


# revision 21
# speedup vs baseline: 1.0386x; 1.0386x over previous
"""CMAttention Trainium2 Bass kernel.

Reference computation (b=2, n=2048, dim=512, H=8 heads, dh=64, M=3 memory tokens):
    q = x @ wq;  k, v = split(x @ wkv);  per-head attention with 3 extra
    memory k/v tokens appended;  out = softmax(q k^T / 8) v;  y = out @ wo + bo.

Sharding: 16 (batch, head) pairs over 8 cores -> each core owns one batch and
two adjacent heads.  Per core everything is local; the out-projection is
row-sharded (per-head) and partial outputs are summed on the host (the
all-reduce of the sharding hint, done at gather time).

Device-side layout (per core, two heads "stacked" on partitions 0-63 / 64-127):
    xt   [4][128, 2048]   x[b]^T in bf16, contraction c on partitions
    qT   [128, 2048]      q^T = wq_s^T-chunks @ xt    (d_global on partitions)
    kT   [128, 2052]      k^T * 1/8 (scale folded into wk on host) ++ memory keys
    v    via PE transpose -> v_aug[h] [128, 17*65]: per j-tile [128, 64+1(ones)]
    scores^T s[j, i] = kT_h^T-slice.T @ qT_h  -> PSUM [128(j), 1024(i)]
      (the two heads' QK matmuls dual-issue on the PE's 64-row groups)
    exp on ScalarE PSUM->SBUF
    av:  out_h^T[65, i] += v_aug_jt.T @ exp_jt   (row 64 = softmax denominator)
    denominator row -> rec_col via tiny PE transposes (no DRAM round-trip)
    out-projection per head + per-partition reciprocal scaling, host sums partials.
"""

import sys

for _p in ("/opt/trn_rl_repo", "/root/.axon_site/_ro/trn_rl_repo"):
    if _p not in sys.path:
        sys.path.insert(0, _p)

import ml_dtypes
import numpy as np

import concourse.bacc as bacc
import concourse.mybir as mybir
import concourse.tile as tile
from concourse import bass_utils
from concourse.masks import make_identity

F32 = mybir.dt.float32
BF = mybir.dt.bfloat16
AF = mybir.ActivationFunctionType
ALU = mybir.AluOpType

H, DH, M = 8, 64, 3
DIM = 512
INNER = H * DH
NSEQ = 2048
B = 2
N_CORES = 8
SCALE = DH ** -0.5
SQRT_M = float(np.sqrt(M))

_CACHE = {}


def _emit(nc, tc, n):
    """Emit the per-core program. n = sequence length (queries)."""
    n_it = n // 128          # i-tiles of 128 queries
    n_jt = n // 128 + 1      # j-tiles: n/128 full + 1 memory tile (3 rows)
    VA = 65                  # v_aug cols per j-tile: 64 dims + ones column

    ap_xt = nc.dram_tensor("xt", [4, 128, n], BF, kind="ExternalInput").ap()
    # weights stored chunk-major along columns: [128, 4*128], col block c =
    # contraction chunk c -> one contiguous 1KB-row DMA instead of 4 small ones
    ap_wq = nc.dram_tensor("wq_s", [128, 512], BF, kind="ExternalInput").ap()
    ap_wk = nc.dram_tensor("wk_s", [128, 512], BF, kind="ExternalInput").ap()
    ap_wv = nc.dram_tensor("wv_s", [128, 512], BF, kind="ExternalInput").ap()
    ap_wo = nc.dram_tensor("wo_s", [128, DIM], BF, kind="ExternalInput").ap()
    ap_mkT = nc.dram_tensor("mkT_s", [128, M], BF, kind="ExternalInput").ap()
    ap_mv = nc.dram_tensor("mv_s", [M, 128], BF, kind="ExternalInput").ap()
    ap_out = nc.dram_tensor("out", [n_it, 128, DIM], BF, kind="ExternalOutput").ap()

    with (
        tc.tile_pool(name="persist", bufs=1) as per,
    ):
        xt = [per.tile([128, n], BF, tag=f"xt{c}", name=f"xt{c}") for c in range(4)]
        wq_all = per.tile([128, 512], BF, tag="wq", name="wq")
        wk_all = per.tile([128, 512], BF, tag="wk", name="wk")
        wv_all = per.tile([128, 512], BF, tag="wv", name="wv")
        wq_sb = [wq_all[:, c * 128 : (c + 1) * 128] for c in range(4)]
        wk_sb = [wk_all[:, c * 128 : (c + 1) * 128] for c in range(4)]
        wv_sb = [wv_all[:, c * 128 : (c + 1) * 128] for c in range(4)]
        wo_sb = per.tile([128, DIM], BF, tag="wo", name="wo")
        qT = per.tile([128, n], BF, tag="qT", name="qT")
        kT = per.tile([128, n + 128], BF, tag="kT", name="kT")
        vT = per.tile([128, n], BF, tag="vT", name="vT")
        v_aug = [per.tile([128, n_jt * VA], BF, tag=f"vaug{h}", name=f"vaug{h}") for h in range(2)]
        oT = per.tile([128, n], BF, tag="oT", name="oT")
        rec_col = per.tile([128, 2 * n_it], F32, tag="rec", name="rec")
        ident = per.tile([128, 128], BF, tag="ident", name="ident")
        ones1 = per.tile([1, 1], F32, tag="ones1", name="ones1")

        # ---- input DMAs: 3 hardware queues (sync/scalar/gpsimd) at ~77B/ns
        # each.  xt split into 16 [128,512] pieces issued round-robin in the
        # k-projection's consumption order (k-chunk outer, c inner) so the
        # PE can start on kT chunk 0 ~2 pieces in.  wk first (needed first),
        # wq early on gpsimd (needed by q0 right after kT chunk 0).
        piece = []
        for k in range(4):
            for c in range(4):
                piece.append(
                    (xt[c][:, k * 512 : (k + 1) * 512],
                     ap_xt[c][:, k * 512 : (k + 1) * 512])
                )
        rest = piece[4:]
        sc = [piece[0], piece[2]] + rest[0::3]
        gp = [(wq_all, ap_wq), piece[1]] + rest[1::3]
        sy = [(wk_all, ap_wk), piece[3]] + rest[2::3]
        for i in range(max(len(sc), len(gp), len(sy))):
            if i < len(sc):
                nc.scalar.dma_start(out=sc[i][0], in_=sc[i][1])
            if i < len(gp):
                nc.gpsimd.dma_start(out=gp[i][0], in_=gp[i][1])
            if i < len(sy):
                nc.sync.dma_start(out=sy[i][0], in_=sy[i][1])
        nc.sync.dma_start(out=wv_all, in_=ap_wv)
        nc.sync.dma_start(out=wo_sb, in_=ap_wo)
        nc.vector.memset(kT[:, n : n + 128], 0.0)
        nc.sync.dma_start(out=kT[:, n : n + M], in_=ap_mkT)
        make_identity(nc, ident[:])
        nc.gpsimd.memset(ones1[:], 1.0)
        for h in range(2):
            nc.vector.memset(v_aug[h][:], 1.0)
            mb = (n_jt - 1) * VA
            nc.vector.memset(v_aug[h][:, mb : mb + VA], 0.0)
            nc.vector.memset(v_aug[h][0:M, mb + 64 : mb + VA], 1.0)
            nc.sync.dma_start(
                out=v_aug[h][0:M, mb : mb + 64],
                in_=ap_mv[:, h * 64 : (h + 1) * 64],
            )

        # ---- minimal pre-attention projections: kT chunk 0 and qT chunk 0
        # only — attention starts as soon as they land; kT chunks 1-3, vT,
        # transposes and remaining qT chunks ride inside the Q0 window as
        # PE filler while the rest of x streams in.
        with tc.tile_pool(name="proj_ps", bufs=4, space="PSUM") as proj_ps:
            for name, w_sb, dst in (("kps0", wk_sb, kT), ("q0ps", wq_sb, qT)):
                ps = proj_ps.tile([128, 512], F32, tag="proj", name=name)
                for c in range(4):
                    nc.tensor.matmul(
                        ps[:],
                        w_sb[c][:],
                        xt[c][:, 0:512],
                        start=(c == 0),
                        stop=(c == 3),
                    )
                nc.scalar.copy(out=dst[:, 0:512], in_=ps[:])

        # ---- attention: i-quarter (512) outer; both heads share one scores
        # PSUM tile (h0 cols 0-511, h1 cols 512-1023) -> one exp call covers
        # both heads; sp double-buffered; av staggered one j-tile behind.
        # The out-projection for quarter q-1 rides inside quarter q's window;
        # its PSUM tiles share the "mix" pool with the av accumulators and
        # deferred q/v projections.
        n_iq = n // 512

        def outproj_quarter(iq, half, mix_pool, stage_pool, tail=False, sp_pool=None):
            ts0 = iq * 4 + (2 if half else 0)
            for t in range(ts0, ts0 + 2):
                p0 = mix_pool.tile([128, 512], F32, tag="mix", name="p0")
                p1 = mix_pool.tile([128, 512], F32, tag="mix", name="p1")
                nc.tensor.matmul(
                    p0[:],
                    oT[0:64, t * 128 : (t + 1) * 128],
                    wo_sb[0:64, :],
                    start=True,
                    stop=True,
                )
                nc.tensor.matmul(
                    p1[:],
                    oT[64:128, t * 128 : (t + 1) * 128],
                    wo_sb[64:128, :],
                    start=True,
                    stop=True,
                )
                a1 = stage_pool.tile([128, 512], F32, tag="a1", name="a1")
                if tail:
                    # ACT is idle after the last exp — do the h1 scale there so
                    # the DVE only runs one op per tile on the critical tail.
                    nc.scalar.activation(
                        out=a1[:],
                        in_=p1[:],
                        func=AF.Copy,
                        scale=rec_col[:, n_it + t : n_it + t + 1],
                    )
                else:
                    nc.vector.tensor_scalar_mul(
                        a1[:], p1[:], rec_col[:, n_it + t : n_it + t + 1]
                    )
                outb = stage_pool.tile([128, 512], BF, tag="outb", name="outb")
                nc.vector.scalar_tensor_tensor(
                    out=outb[:],
                    in0=p0[:],
                    scalar=rec_col[:, t : t + 1],
                    in1=a1[:],
                    op0=ALU.mult,
                    op1=ALU.add,
                )
                if tail:
                    eng = (nc.sync, nc.gpsimd, nc.scalar)[t % 3]
                else:
                    eng = nc.sync if t % 2 == 0 else nc.gpsimd
                eng.dma_start(out=ap_out[t], in_=outb[:])

        def kproj_chunk(k, mix_pool):
            kps = mix_pool.tile([128, 512], F32, tag="mix", name=f"kps{k}")
            for c in range(4):
                nc.tensor.matmul(
                    kps[:],
                    wk_sb[c][:],
                    xt[c][:, k * 512 : (k + 1) * 512],
                    start=(c == 0),
                    stop=(c == 3),
                )
            nc.scalar.copy(out=kT[:, k * 512 : (k + 1) * 512], in_=kps[:])

        with (
            tc.tile_pool(name="s_ps", bufs=2, space="PSUM") as s_ps_pool,
            tc.tile_pool(name="mix_ps", bufs=4, space="PSUM") as mix_ps,
            tc.tile_pool(name="exp_sb", bufs=11) as exp_pool,
            tc.tile_pool(name="small", bufs=4) as small,
            tc.tile_pool(name="ostage", bufs=4) as ostage,
        ):
            def do_rec(riq, dens):
                for h in range(2):
                    dc = mix_ps.tile([128, 4], F32, tag="mix", name="dc")
                    for t in range(4):
                        nc.tensor.transpose(
                            dc[:, t : t + 1],
                            dens[h][0:1, t * 128 : (t + 1) * 128],
                            ones1[:],
                        )
                    nc.vector.reciprocal(
                        out=rec_col[:, h * n_it + riq * 4 : h * n_it + riq * 4 + 4],
                        in_=dc[:],
                    )

            deferred_rec = None
            for iq in range(n_iq):
                i0 = iq * 512
                avs = [
                    mix_ps.tile([VA, 512], F32, tag="mix", name=f"av{h}")
                    for h in range(2)
                ]
                pending = []
                for jt in range(n_jt):
                    sp = s_ps_pool.tile([128, 1024], F32, tag="sp", name="sp")
                    for h in range(2):
                        hp = h * 64
                        nc.tensor.matmul(
                            sp[:, h * 512 : (h + 1) * 512],
                            kT[hp : hp + 64, jt * 128 : (jt + 1) * 128],
                            qT[hp : hp + 64, i0 : i0 + 512],
                            start=True,
                            stop=True,
                        )
                    et = exp_pool.tile([128, 1024], BF, tag="exp", name="et")
                    nc.scalar.activation(out=et[:], in_=sp[:], func=AF.Exp)
                    pending.append((et, jt))
                    if jt == 1 and deferred_rec is not None:
                        do_rec(*deferred_rec)
                        deferred_rec = None
                    if iq == 0 and jt in (2, 4, 6):
                        # kT chunks 1-3 ride just behind the arriving x stream
                        kproj_chunk(jt // 2, mix_ps)
                    if iq == 0 and jt in (7, 9):
                        icp = 0 if jt == 7 else 2
                        vps = [
                            mix_ps.tile([128, 512], F32, tag="mix", name="vps")
                            for _ in range(2)
                        ]
                        for c in range(4):
                            for k in range(2):
                                nc.tensor.matmul(
                                    vps[k][:],
                                    wv_sb[c][:],
                                    xt[c][:, (icp + k) * 512 : (icp + k + 1) * 512],
                                    start=(c == 0),
                                    stop=(c == 3),
                                )
                        for k in range(2):
                            nc.vector.tensor_copy(
                                out=vT[:, (icp + k) * 512 : (icp + k + 1) * 512],
                                in_=vps[k][:],
                            )
                    if iq == 0 and jt in (8, 10):
                        t0 = 0 if jt == 8 else 8
                        for tjt in range(t0, t0 + 8):
                            pt = mix_ps.tile([128, 128], BF, tag="mix", name="tr")
                            nc.tensor.transpose(
                                pt[:], vT[:, tjt * 128 : (tjt + 1) * 128], ident[:]
                            )
                            for h in range(2):
                                nc.vector.tensor_copy(
                                    out=v_aug[h][:, tjt * VA : tjt * VA + 64],
                                    in_=pt[:, h * 64 : (h + 1) * 64],
                                )
                    if iq == 0:
                        n_pop = 0
                        if jt >= 9:
                            n_pop = 2 if len(pending) > 6 else 1
                    else:
                        n_pop = 1 if len(pending) > 2 else 0
                    for _ in range(n_pop):
                        pet, pjt = pending.pop(0)
                        for h in range(2):
                            nc.tensor.matmul(
                                avs[h][:],
                                v_aug[h][:, pjt * VA : (pjt + 1) * VA],
                                pet[:, h * 512 : (h + 1) * 512],
                                start=(pjt == 0),
                                stop=False,
                            )
                    if iq == 0 and jt in (11, 13, 15):
                        # deferred qT chunk, one per insertion point
                        ic = (jt - 11) // 2 + 1
                        qp = mix_ps.tile([128, 512], F32, tag="mix", name="qdef")
                        for c in range(4):
                            nc.tensor.matmul(
                                qp[:],
                                wq_sb[c][:],
                                xt[c][:, ic * 512 : (ic + 1) * 512],
                                start=(c == 0),
                                stop=(c == 3),
                            )
                        nc.vector.tensor_copy(
                            out=qT[:, ic * 512 : (ic + 1) * 512], in_=qp[:]
                        )
                    if jt in (4, 8) and iq >= 1:
                        # out-projection for the previous quarter rides here
                        # (2 tiles per insertion), round-trip long completed
                        outproj_quarter(iq - 1, jt == 8, mix_ps, ostage)
                while pending:
                    pet, pjt = pending.pop(0)
                    for h in range(2):
                        nc.tensor.matmul(
                            avs[h][:],
                            v_aug[h][:, pjt * VA : (pjt + 1) * VA],
                            pet[:, h * 512 : (h + 1) * 512],
                            start=(pjt == 0),
                            stop=(pjt == n_jt - 1),
                        )
                # epilogue for this i-quarter: pull the denominator rows and
                # oT out of PSUM; the rec computation (PE transposes + recip)
                # is deferred into the next quarter's window so the PE never
                # idles at the boundary.  DVE does the copies mid-run; ACT
                # (idle after the last exp) takes them on the final quarter.
                dens = []
                for h in range(2):
                    den = small.tile([1, 512], F32, tag="den", name="den")
                    if iq == n_iq - 1:
                        nc.scalar.copy(out=den[:], in_=avs[h][64:65, :])
                    else:
                        nc.vector.tensor_copy(out=den[:], in_=avs[h][64:65, :])
                    dens.append(den)
                for h in range(2):
                    hp = h * 64
                    nc.vector.tensor_copy(
                        out=oT[hp : hp + 64, i0 : i0 + 512], in_=avs[h][0:64, :]
                    )
                deferred_rec = (iq, dens)
                if iq == n_iq - 1:
                    do_rec(*deferred_rec)
            # final quarter's out-projection (ACT helps with the scaling)
            outproj_quarter(n_iq - 1, False, mix_ps, ostage, tail=True, sp_pool=s_ps_pool)
            outproj_quarter(n_iq - 1, True, mix_ps, ostage, tail=True, sp_pool=s_ps_pool)


def _build(n=NSEQ):
    if n in _CACHE:
        return _CACHE[n]
    nc = bacc.Bacc("TRN2", debug=False, num_devices=N_CORES)
    with tile.TileContext(nc) as tc:
        _emit(nc, tc, n)
    nc.compile()
    _CACHE[n] = nc
    return nc


def _prep_in_maps(x, wq, wkv, wo, m_k, m_v, n):
    x = np.asarray(x, np.float32)
    wq = np.asarray(wq, np.float32)
    wkv = np.asarray(wkv, np.float32)
    wo = np.asarray(wo, np.float32)
    m_k = np.asarray(m_k, np.float32)
    m_v = np.asarray(m_v, np.float32)

    wk = wkv[:, :INNER]
    wv = wkv[:, INNER:]
    # memory tokens: flat reshape (M, INNER) -> (H, M, DH), exactly as reference
    mk_heads = m_k.reshape(M * INNER).reshape(H, M, DH)  # * SQRT_DH * SCALE == 1.0
    mv_heads = m_v.reshape(M * INNER).reshape(H, M, DH) * SQRT_M

    in_maps = []
    for cid in range(N_CORES):
        b = cid // 4
        h0 = 2 * (cid % 4)
        sl = slice(h0 * DH, (h0 + 2) * DH)
        in_maps.append(
            {
                "xt": np.ascontiguousarray(x[b].T)
                .reshape(4, 128, n)
                .astype(ml_dtypes.bfloat16),
                "wq_s": np.ascontiguousarray(
                    wq[:, sl].reshape(4, 128, 128).transpose(1, 0, 2).reshape(128, 512)
                ).astype(ml_dtypes.bfloat16),
                "wk_s": np.ascontiguousarray(
                    (wk[:, sl] * SCALE)
                    .reshape(4, 128, 128)
                    .transpose(1, 0, 2)
                    .reshape(128, 512)
                ).astype(ml_dtypes.bfloat16),
                "wv_s": np.ascontiguousarray(
                    wv[:, sl].reshape(4, 128, 128).transpose(1, 0, 2).reshape(128, 512)
                ).astype(ml_dtypes.bfloat16),
                "wo_s": np.ascontiguousarray(wo[sl, :]).astype(ml_dtypes.bfloat16),
                "mkT_s": np.ascontiguousarray(
                    np.concatenate([mk_heads[h0].T, mk_heads[h0 + 1].T], axis=0)
                ).astype(ml_dtypes.bfloat16),
                "mv_s": np.ascontiguousarray(
                    np.concatenate([mv_heads[h0], mv_heads[h0 + 1]], axis=1)
                ).astype(ml_dtypes.bfloat16),
            }
        )
    return in_maps


def _gather(results, bo, n):
    bo = np.asarray(bo, np.float32)
    out = np.zeros((B, n, DIM), np.float32)
    for cid in range(N_CORES):
        out[cid // 4] += results[cid]["out"].reshape(n, DIM).astype(np.float32)
    out += bo
    return out


def run(x, wq, wkv, wo, bo, m_k, m_v, trace=False, n=NSEQ):
    nc = _build(n)
    in_maps = _prep_in_maps(x, wq, wkv, wo, m_k, m_v, n)
    res = bass_utils.run_bass_kernel_spmd(
        nc, in_maps, core_ids=list(range(N_CORES)), trace=trace
    )
    return _gather(res.results, bo, n), res


def kernel(x, wq, wkv, wo, bo, m_k, m_v):
    out, _ = run(x, wq, wkv, wo, bo, m_k, m_v)
    return out


# revision 22
# speedup vs baseline: 1.0462x; 1.0073x over previous
"""CMAttention Trainium2 Bass kernel.

Reference computation (b=2, n=2048, dim=512, H=8 heads, dh=64, M=3 memory tokens):
    q = x @ wq;  k, v = split(x @ wkv);  per-head attention with 3 extra
    memory k/v tokens appended;  out = softmax(q k^T / 8) v;  y = out @ wo + bo.

Sharding: 16 (batch, head) pairs over 8 cores -> each core owns one batch and
two adjacent heads.  Per core everything is local; the out-projection is
row-sharded (per-head) and partial outputs are summed on the host (the
all-reduce of the sharding hint, done at gather time).

Device-side layout (per core, two heads "stacked" on partitions 0-63 / 64-127):
    xt   [4][128, 2048]   x[b]^T in bf16, contraction c on partitions
    qT   [128, 2048]      q^T = wq_s^T-chunks @ xt    (d_global on partitions)
    kT   [128, 2052]      k^T * 1/8 (scale folded into wk on host) ++ memory keys
    v    via PE transpose -> v_aug[h] [128, 17*65]: per j-tile [128, 64+1(ones)]
    scores^T s[j, i] = kT_h^T-slice.T @ qT_h  -> PSUM [128(j), 1024(i)]
      (the two heads' QK matmuls dual-issue on the PE's 64-row groups)
    exp on ScalarE PSUM->SBUF
    av:  out_h^T[65, i] += v_aug_jt.T @ exp_jt   (row 64 = softmax denominator)
    denominator row -> rec_col via tiny PE transposes (no DRAM round-trip)
    out-projection per head + per-partition reciprocal scaling, host sums partials.
"""

import sys

for _p in ("/opt/trn_rl_repo", "/root/.axon_site/_ro/trn_rl_repo"):
    if _p not in sys.path:
        sys.path.insert(0, _p)

import ml_dtypes
import numpy as np

import concourse.bacc as bacc
import concourse.mybir as mybir
import concourse.tile as tile
from concourse import bass_utils
from concourse.masks import make_identity

F32 = mybir.dt.float32
BF = mybir.dt.bfloat16
AF = mybir.ActivationFunctionType
ALU = mybir.AluOpType

H, DH, M = 8, 64, 3
DIM = 512
INNER = H * DH
NSEQ = 2048
B = 2
N_CORES = 8
SCALE = DH ** -0.5
SQRT_M = float(np.sqrt(M))

_CACHE = {}


def _emit(nc, tc, n):
    """Emit the per-core program. n = sequence length (queries)."""
    n_it = n // 128          # i-tiles of 128 queries
    n_jt = n // 128 + 1      # j-tiles: n/128 full + 1 memory tile (3 rows)
    VA = 65                  # v_aug cols per j-tile: 64 dims + ones column

    ap_xt = nc.dram_tensor("xt", [4, 128, n], BF, kind="ExternalInput").ap()
    # weights stored chunk-major along columns: [128, 4*128], col block c =
    # contraction chunk c -> one contiguous 1KB-row DMA instead of 4 small ones
    ap_wq = nc.dram_tensor("wq_s", [128, 512], BF, kind="ExternalInput").ap()
    ap_wk = nc.dram_tensor("wk_s", [128, 512], BF, kind="ExternalInput").ap()
    ap_wv = nc.dram_tensor("wv_s", [128, 512], BF, kind="ExternalInput").ap()
    ap_wo = nc.dram_tensor("wo_s", [128, DIM], BF, kind="ExternalInput").ap()
    ap_mkT = nc.dram_tensor("mkT_s", [128, M], BF, kind="ExternalInput").ap()
    ap_mv = nc.dram_tensor("mv_s", [M, 128], BF, kind="ExternalInput").ap()
    ap_out = nc.dram_tensor("out", [n_it, 128, DIM], BF, kind="ExternalOutput").ap()

    with (
        tc.tile_pool(name="persist", bufs=1) as per,
    ):
        xt = [per.tile([128, n], BF, tag=f"xt{c}", name=f"xt{c}") for c in range(4)]
        wq_all = per.tile([128, 512], BF, tag="wq", name="wq")
        wk_all = per.tile([128, 512], BF, tag="wk", name="wk")
        wv_all = per.tile([128, 512], BF, tag="wv", name="wv")
        wq_sb = [wq_all[:, c * 128 : (c + 1) * 128] for c in range(4)]
        wk_sb = [wk_all[:, c * 128 : (c + 1) * 128] for c in range(4)]
        wv_sb = [wv_all[:, c * 128 : (c + 1) * 128] for c in range(4)]
        wo_sb = per.tile([128, DIM], BF, tag="wo", name="wo")
        qT = per.tile([128, n], BF, tag="qT", name="qT")
        kT = per.tile([128, n + 128], BF, tag="kT", name="kT")
        vT = per.tile([128, n], BF, tag="vT", name="vT")
        v_aug = [per.tile([128, n_jt * VA], BF, tag=f"vaug{h}", name=f"vaug{h}") for h in range(2)]
        oT = per.tile([128, n], BF, tag="oT", name="oT")
        rec_col = per.tile([128, 2 * n_it], F32, tag="rec", name="rec")
        ident = per.tile([128, 128], BF, tag="ident", name="ident")
        ones1 = per.tile([1, 1], F32, tag="ones1", name="ones1")

        # ---- input DMAs: 3 hardware queues (sync/scalar/gpsimd) at ~77B/ns
        # each.  xt split into 16 [128,512] pieces issued round-robin in the
        # k-projection's consumption order (k-chunk outer, c inner) so the
        # PE can start on kT chunk 0 ~2 pieces in.  wk first (needed first),
        # wq early on gpsimd (needed by q0 right after kT chunk 0).
        piece = []
        for k in range(4):
            for c in range(4):
                piece.append(
                    (xt[c][:, k * 512 : (k + 1) * 512],
                     ap_xt[c][:, k * 512 : (k + 1) * 512])
                )
        rest = piece[4:]
        sc = [piece[0], piece[2]] + rest[0::3]
        gp = [piece[1], (wq_all, ap_wq)] + rest[1::3]
        sy = [(wk_all, ap_wk), piece[3]] + rest[2::3]
        for i in range(max(len(sc), len(gp), len(sy))):
            if i < len(sc):
                nc.scalar.dma_start(out=sc[i][0], in_=sc[i][1])
            if i < len(gp):
                nc.gpsimd.dma_start(out=gp[i][0], in_=gp[i][1])
            if i < len(sy):
                nc.sync.dma_start(out=sy[i][0], in_=sy[i][1])
        nc.sync.dma_start(out=wv_all, in_=ap_wv)
        nc.sync.dma_start(out=wo_sb, in_=ap_wo)
        nc.vector.memset(kT[:, n : n + 128], 0.0)
        nc.sync.dma_start(out=kT[:, n : n + M], in_=ap_mkT)
        make_identity(nc, ident[:])
        nc.gpsimd.memset(ones1[:], 1.0)
        for h in range(2):
            nc.vector.memset(v_aug[h][:], 1.0)
            mb = (n_jt - 1) * VA
            nc.vector.memset(v_aug[h][:, mb : mb + VA], 0.0)
            nc.vector.memset(v_aug[h][0:M, mb + 64 : mb + VA], 1.0)
            nc.sync.dma_start(
                out=v_aug[h][0:M, mb : mb + 64],
                in_=ap_mv[:, h * 64 : (h + 1) * 64],
            )

        # ---- minimal pre-attention projections: kT chunk 0 and qT chunk 0
        # only — attention starts as soon as they land; kT chunks 1-3, vT,
        # transposes and remaining qT chunks ride inside the Q0 window as
        # PE filler while the rest of x streams in.
        with tc.tile_pool(name="proj_ps", bufs=4, space="PSUM") as proj_ps:
            for name, w_sb, dst in (("kps0", wk_sb, kT), ("q0ps", wq_sb, qT)):
                ps = proj_ps.tile([128, 512], F32, tag="proj", name=name)
                for c in range(4):
                    nc.tensor.matmul(
                        ps[:],
                        w_sb[c][:],
                        xt[c][:, 0:512],
                        start=(c == 0),
                        stop=(c == 3),
                    )
                nc.scalar.copy(out=dst[:, 0:512], in_=ps[:])

        # ---- attention: i-quarter (512) outer; both heads share one scores
        # PSUM tile (h0 cols 0-511, h1 cols 512-1023) -> one exp call covers
        # both heads; sp double-buffered; av staggered one j-tile behind.
        # The out-projection for quarter q-1 rides inside quarter q's window;
        # its PSUM tiles share the "mix" pool with the av accumulators and
        # deferred q/v projections.
        n_iq = n // 512

        def outproj_quarter(iq, half, mix_pool, stage_pool, tail=False, sp_pool=None):
            ts0 = iq * 4 + (2 if half else 0)
            for t in range(ts0, ts0 + 2):
                if tail and t % 2 == 1:
                    p01 = sp_pool.tile([128, 1024], F32, tag="sp", name="p01")
                    p0 = p01[:, 0:512]
                    p1 = p01[:, 512:1024]
                else:
                    p0 = mix_pool.tile([128, 512], F32, tag="mix", name="p0")
                    p1 = mix_pool.tile([128, 512], F32, tag="mix", name="p1")
                nc.tensor.matmul(
                    p0[:],
                    oT[0:64, t * 128 : (t + 1) * 128],
                    wo_sb[0:64, :],
                    start=True,
                    stop=True,
                )
                nc.tensor.matmul(
                    p1[:],
                    oT[64:128, t * 128 : (t + 1) * 128],
                    wo_sb[64:128, :],
                    start=True,
                    stop=True,
                )
                a1 = stage_pool.tile([128, 512], F32, tag="a1", name="a1")
                if tail:
                    # ACT is idle after the last exp — do the h1 scale there so
                    # the DVE only runs one op per tile on the critical tail.
                    nc.scalar.activation(
                        out=a1[:],
                        in_=p1[:],
                        func=AF.Copy,
                        scale=rec_col[:, n_it + t : n_it + t + 1],
                    )
                else:
                    nc.vector.tensor_scalar_mul(
                        a1[:], p1[:], rec_col[:, n_it + t : n_it + t + 1]
                    )
                outb = stage_pool.tile([128, 512], BF, tag="outb", name="outb")
                nc.vector.scalar_tensor_tensor(
                    out=outb[:],
                    in0=p0[:],
                    scalar=rec_col[:, t : t + 1],
                    in1=a1[:],
                    op0=ALU.mult,
                    op1=ALU.add,
                )
                if tail:
                    eng = (nc.sync, nc.gpsimd, nc.scalar)[t % 3]
                else:
                    eng = nc.sync if t % 2 == 0 else nc.gpsimd
                eng.dma_start(out=ap_out[t], in_=outb[:])

        def kproj_chunk(k, mix_pool):
            kps = mix_pool.tile([128, 512], F32, tag="mix", name=f"kps{k}")
            for c in range(4):
                nc.tensor.matmul(
                    kps[:],
                    wk_sb[c][:],
                    xt[c][:, k * 512 : (k + 1) * 512],
                    start=(c == 0),
                    stop=(c == 3),
                )
            nc.scalar.copy(out=kT[:, k * 512 : (k + 1) * 512], in_=kps[:])

        with (
            tc.tile_pool(name="s_ps", bufs=2, space="PSUM") as s_ps_pool,
            tc.tile_pool(name="mix_ps", bufs=4, space="PSUM") as mix_ps,
            tc.tile_pool(name="exp_sb", bufs=11) as exp_pool,
            tc.tile_pool(name="small", bufs=4) as small,
            tc.tile_pool(name="ostage", bufs=4) as ostage,
        ):
            def do_rec(riq, dens):
                for h in range(2):
                    dc = mix_ps.tile([128, 4], F32, tag="mix", name="dc")
                    for t in range(4):
                        nc.tensor.transpose(
                            dc[:, t : t + 1],
                            dens[h][0:1, t * 128 : (t + 1) * 128],
                            ones1[:],
                        )
                    nc.vector.reciprocal(
                        out=rec_col[:, h * n_it + riq * 4 : h * n_it + riq * 4 + 4],
                        in_=dc[:],
                    )

            deferred_rec = None
            for iq in range(n_iq):
                i0 = iq * 512
                avs = [
                    mix_ps.tile([VA, 512], F32, tag="mix", name=f"av{h}")
                    for h in range(2)
                ]
                pending = []
                for jt in range(n_jt):
                    sp = s_ps_pool.tile([128, 1024], F32, tag="sp", name="sp")
                    for h in range(2):
                        hp = h * 64
                        nc.tensor.matmul(
                            sp[:, h * 512 : (h + 1) * 512],
                            kT[hp : hp + 64, jt * 128 : (jt + 1) * 128],
                            qT[hp : hp + 64, i0 : i0 + 512],
                            start=True,
                            stop=True,
                        )
                    et = exp_pool.tile([128, 1024], BF, tag="exp", name="et")
                    nc.scalar.activation(out=et[:], in_=sp[:], func=AF.Exp)
                    pending.append((et, jt))
                    if jt == 1 and deferred_rec is not None:
                        do_rec(*deferred_rec)
                        deferred_rec = None
                    if iq == 0 and jt in (2, 4, 6):
                        # kT chunks 1-3 ride just behind the arriving x stream
                        kproj_chunk(jt // 2, mix_ps)
                    if iq == 0 and jt in (7, 9):
                        icp = 0 if jt == 7 else 2
                        vps = [
                            mix_ps.tile([128, 512], F32, tag="mix", name="vps")
                            for _ in range(2)
                        ]
                        for c in range(4):
                            for k in range(2):
                                nc.tensor.matmul(
                                    vps[k][:],
                                    wv_sb[c][:],
                                    xt[c][:, (icp + k) * 512 : (icp + k + 1) * 512],
                                    start=(c == 0),
                                    stop=(c == 3),
                                )
                        for k in range(2):
                            nc.vector.tensor_copy(
                                out=vT[:, (icp + k) * 512 : (icp + k + 1) * 512],
                                in_=vps[k][:],
                            )
                    if iq == 0 and jt in (8, 10):
                        t0 = 0 if jt == 8 else 8
                        for tjt in range(t0, t0 + 8):
                            pt = mix_ps.tile([128, 128], BF, tag="mix", name="tr")
                            nc.tensor.transpose(
                                pt[:], vT[:, tjt * 128 : (tjt + 1) * 128], ident[:]
                            )
                            for h in range(2):
                                nc.vector.tensor_copy(
                                    out=v_aug[h][:, tjt * VA : tjt * VA + 64],
                                    in_=pt[:, h * 64 : (h + 1) * 64],
                                )
                    if iq == 0:
                        n_pop = 0
                        if jt >= 9:
                            n_pop = 2 if len(pending) > 6 else 1
                    else:
                        n_pop = 1 if len(pending) > 2 else 0
                    for _ in range(n_pop):
                        pet, pjt = pending.pop(0)
                        for h in range(2):
                            nc.tensor.matmul(
                                avs[h][:],
                                v_aug[h][:, pjt * VA : (pjt + 1) * VA],
                                pet[:, h * 512 : (h + 1) * 512],
                                start=(pjt == 0),
                                stop=False,
                            )
                    if iq == 0 and jt in (11, 13, 15):
                        # deferred qT chunk, one per insertion point
                        ic = (jt - 11) // 2 + 1
                        qp = mix_ps.tile([128, 512], F32, tag="mix", name="qdef")
                        for c in range(4):
                            nc.tensor.matmul(
                                qp[:],
                                wq_sb[c][:],
                                xt[c][:, ic * 512 : (ic + 1) * 512],
                                start=(c == 0),
                                stop=(c == 3),
                            )
                        nc.vector.tensor_copy(
                            out=qT[:, ic * 512 : (ic + 1) * 512], in_=qp[:]
                        )
                    if jt in (4, 8) and iq >= 1:
                        # out-projection for the previous quarter rides here
                        # (2 tiles per insertion), round-trip long completed
                        outproj_quarter(iq - 1, jt == 8, mix_ps, ostage)
                while pending:
                    pet, pjt = pending.pop(0)
                    for h in range(2):
                        nc.tensor.matmul(
                            avs[h][:],
                            v_aug[h][:, pjt * VA : (pjt + 1) * VA],
                            pet[:, h * 512 : (h + 1) * 512],
                            start=(pjt == 0),
                            stop=(pjt == n_jt - 1),
                        )
                # epilogue for this i-quarter: pull the denominator rows and
                # oT out of PSUM; the rec computation (PE transposes + recip)
                # is deferred into the next quarter's window so the PE never
                # idles at the boundary.  DVE does the copies mid-run; ACT
                # (idle after the last exp) takes them on the final quarter.
                dens = []
                for h in range(2):
                    den = small.tile([1, 512], F32, tag="den", name="den")
                    if iq == n_iq - 1 and h == 0:
                        nc.scalar.copy(out=den[:], in_=avs[h][64:65, :])
                    else:
                        nc.vector.tensor_copy(out=den[:], in_=avs[h][64:65, :])
                    dens.append(den)
                for h in range(2):
                    hp = h * 64
                    nc.vector.tensor_copy(
                        out=oT[hp : hp + 64, i0 : i0 + 512], in_=avs[h][0:64, :]
                    )
                deferred_rec = (iq, dens)
                if iq == n_iq - 1:
                    do_rec(*deferred_rec)
            # final quarter's out-projection (ACT helps with the scaling)
            outproj_quarter(n_iq - 1, False, mix_ps, ostage, tail=True, sp_pool=s_ps_pool)
            outproj_quarter(n_iq - 1, True, mix_ps, ostage, tail=True, sp_pool=s_ps_pool)


def _build(n=NSEQ):
    if n in _CACHE:
        return _CACHE[n]
    nc = bacc.Bacc("TRN2", debug=False, num_devices=N_CORES)
    with tile.TileContext(nc) as tc:
        _emit(nc, tc, n)
    nc.compile()
    _CACHE[n] = nc
    return nc


def _prep_in_maps(x, wq, wkv, wo, m_k, m_v, n):
    x = np.asarray(x, np.float32)
    wq = np.asarray(wq, np.float32)
    wkv = np.asarray(wkv, np.float32)
    wo = np.asarray(wo, np.float32)
    m_k = np.asarray(m_k, np.float32)
    m_v = np.asarray(m_v, np.float32)

    wk = wkv[:, :INNER]
    wv = wkv[:, INNER:]
    # memory tokens: flat reshape (M, INNER) -> (H, M, DH), exactly as reference
    mk_heads = m_k.reshape(M * INNER).reshape(H, M, DH)  # * SQRT_DH * SCALE == 1.0
    mv_heads = m_v.reshape(M * INNER).reshape(H, M, DH) * SQRT_M

    in_maps = []
    for cid in range(N_CORES):
        b = cid // 4
        h0 = 2 * (cid % 4)
        sl = slice(h0 * DH, (h0 + 2) * DH)
        in_maps.append(
            {
                "xt": np.ascontiguousarray(x[b].T)
                .reshape(4, 128, n)
                .astype(ml_dtypes.bfloat16),
                "wq_s": np.ascontiguousarray(
                    wq[:, sl].reshape(4, 128, 128).transpose(1, 0, 2).reshape(128, 512)
                ).astype(ml_dtypes.bfloat16),
                "wk_s": np.ascontiguousarray(
                    (wk[:, sl] * SCALE)
                    .reshape(4, 128, 128)
                    .transpose(1, 0, 2)
                    .reshape(128, 512)
                ).astype(ml_dtypes.bfloat16),
                "wv_s": np.ascontiguousarray(
                    wv[:, sl].reshape(4, 128, 128).transpose(1, 0, 2).reshape(128, 512)
                ).astype(ml_dtypes.bfloat16),
                "wo_s": np.ascontiguousarray(wo[sl, :]).astype(ml_dtypes.bfloat16),
                "mkT_s": np.ascontiguousarray(
                    np.concatenate([mk_heads[h0].T, mk_heads[h0 + 1].T], axis=0)
                ).astype(ml_dtypes.bfloat16),
                "mv_s": np.ascontiguousarray(
                    np.concatenate([mv_heads[h0], mv_heads[h0 + 1]], axis=1)
                ).astype(ml_dtypes.bfloat16),
            }
        )
    return in_maps


def _gather(results, bo, n):
    bo = np.asarray(bo, np.float32)
    out = np.zeros((B, n, DIM), np.float32)
    for cid in range(N_CORES):
        out[cid // 4] += results[cid]["out"].reshape(n, DIM).astype(np.float32)
    out += bo
    return out


def run(x, wq, wkv, wo, bo, m_k, m_v, trace=False, n=NSEQ):
    nc = _build(n)
    in_maps = _prep_in_maps(x, wq, wkv, wo, m_k, m_v, n)
    res = bass_utils.run_bass_kernel_spmd(
        nc, in_maps, core_ids=list(range(N_CORES)), trace=trace
    )
    return _gather(res.results, bo, n), res


def kernel(x, wq, wkv, wo, bo, m_k, m_v):
    out, _ = run(x, wq, wkv, wo, bo, m_k, m_v)
    return out


# revision 24
# speedup vs baseline: 1.0544x; 1.0078x over previous
"""CMAttention Trainium2 Bass kernel.

Reference computation (b=2, n=2048, dim=512, H=8 heads, dh=64, M=3 memory tokens):
    q = x @ wq;  k, v = split(x @ wkv);  per-head attention with 3 extra
    memory k/v tokens appended;  out = softmax(q k^T / 8) v;  y = out @ wo + bo.

Sharding: 16 (batch, head) pairs over 8 cores -> each core owns one batch and
two adjacent heads.  Per core everything is local; the out-projection is
row-sharded (per-head) and partial outputs are summed on the host (the
all-reduce of the sharding hint, done at gather time).

Device-side layout (per core, two heads "stacked" on partitions 0-63 / 64-127):
    xt   [4][128, 2048]   x[b]^T in bf16, contraction c on partitions
    qT   [128, 2048]      q^T = wq_s^T-chunks @ xt    (d_global on partitions)
    kT   [128, 2052]      k^T * 1/8 (scale folded into wk on host) ++ memory keys
    v    via PE transpose -> v_aug[h] [128, 17*65]: per j-tile [128, 64+1(ones)]
    scores^T s[j, i] = kT_h^T-slice.T @ qT_h  -> PSUM [128(j), 1024(i)]
      (the two heads' QK matmuls dual-issue on the PE's 64-row groups)
    exp on ScalarE PSUM->SBUF
    av:  out_h^T[65, i] += v_aug_jt.T @ exp_jt   (row 64 = softmax denominator)
    denominator row -> rec_col via tiny PE transposes (no DRAM round-trip)
    out-projection per head + per-partition reciprocal scaling, host sums partials.
"""

import sys

for _p in ("/opt/trn_rl_repo", "/root/.axon_site/_ro/trn_rl_repo"):
    if _p not in sys.path:
        sys.path.insert(0, _p)

import ml_dtypes
import numpy as np

import concourse.bacc as bacc
import concourse.mybir as mybir
import concourse.tile as tile
from concourse import bass_utils
from concourse.masks import make_identity

F32 = mybir.dt.float32
BF = mybir.dt.bfloat16
AF = mybir.ActivationFunctionType
ALU = mybir.AluOpType

H, DH, M = 8, 64, 3
DIM = 512
INNER = H * DH
NSEQ = 2048
B = 2
N_CORES = 8
SCALE = DH ** -0.5
SQRT_M = float(np.sqrt(M))

_CACHE = {}


def _emit(nc, tc, n):
    """Emit the per-core program. n = sequence length (queries)."""
    n_it = n // 128          # i-tiles of 128 queries
    n_jt = n // 128 + 1      # j-tiles: n/128 full + 1 memory tile (3 rows)
    VA = 65                  # v_aug cols per j-tile: 64 dims + ones column

    ap_xt = nc.dram_tensor("xt", [4, 128, n], BF, kind="ExternalInput").ap()
    # weights stored chunk-major along columns: [128, 4*128], col block c =
    # contraction chunk c -> one contiguous 1KB-row DMA instead of 4 small ones
    ap_wq = nc.dram_tensor("wq_s", [128, 512], BF, kind="ExternalInput").ap()
    ap_wk = nc.dram_tensor("wk_s", [128, 512], BF, kind="ExternalInput").ap()
    ap_wv = nc.dram_tensor("wv_s", [128, 512], BF, kind="ExternalInput").ap()
    ap_wo = nc.dram_tensor("wo_s", [128, DIM], BF, kind="ExternalInput").ap()
    ap_mkT = nc.dram_tensor("mkT_s", [128, M], BF, kind="ExternalInput").ap()
    ap_mv = nc.dram_tensor("mv_s", [M, 128], BF, kind="ExternalInput").ap()
    ap_out = nc.dram_tensor("out", [n_it, 128, DIM], BF, kind="ExternalOutput").ap()

    with (
        tc.tile_pool(name="persist", bufs=1) as per,
    ):
        xt = [per.tile([128, n], BF, tag=f"xt{c}", name=f"xt{c}") for c in range(4)]
        wq_all = per.tile([128, 512], BF, tag="wq", name="wq")
        wk_all = per.tile([128, 512], BF, tag="wk", name="wk")
        wv_all = per.tile([128, 512], BF, tag="wv", name="wv")
        wq_sb = [wq_all[:, c * 128 : (c + 1) * 128] for c in range(4)]
        wk_sb = [wk_all[:, c * 128 : (c + 1) * 128] for c in range(4)]
        wv_sb = [wv_all[:, c * 128 : (c + 1) * 128] for c in range(4)]
        wo_sb = per.tile([128, DIM], BF, tag="wo", name="wo")
        qT = per.tile([128, n], BF, tag="qT", name="qT")
        kT = per.tile([128, n + 128], BF, tag="kT", name="kT")
        vT = per.tile([128, n], BF, tag="vT", name="vT")
        v_aug = [per.tile([128, n_jt * VA], BF, tag=f"vaug{h}", name=f"vaug{h}") for h in range(2)]
        oT = per.tile([128, n], BF, tag="oT", name="oT")
        rec_col = per.tile([128, 2 * n_it], F32, tag="rec", name="rec")
        ident = per.tile([128, 128], BF, tag="ident", name="ident")
        ones1 = per.tile([1, 1], F32, tag="ones1", name="ones1")

        # ---- input DMAs: 3 hardware queues (sync/scalar/gpsimd) at ~77B/ns
        # each.  xt split into 16 [128,512] pieces issued round-robin in the
        # k-projection's consumption order (k-chunk outer, c inner) so the
        # PE can start on kT chunk 0 ~2 pieces in.  wk first (needed first),
        # wq early on gpsimd (needed by q0 right after kT chunk 0).
        piece = []
        for k in range(4):
            for c in range(4):
                piece.append(
                    (xt[c][:, k * 512 : (k + 1) * 512],
                     ap_xt[c][:, k * 512 : (k + 1) * 512])
                )
        rest = piece[4:]
        sc = [piece[0], piece[2]] + rest[0::3]
        gp = [piece[1], (wq_all, ap_wq)] + rest[1::3]
        sy = [(wk_all, ap_wk), piece[3]] + rest[2::3]
        for i in range(max(len(sc), len(gp), len(sy))):
            if i < len(sc):
                nc.scalar.dma_start(out=sc[i][0], in_=sc[i][1])
            if i < len(gp):
                nc.gpsimd.dma_start(out=gp[i][0], in_=gp[i][1])
            if i < len(sy):
                nc.sync.dma_start(out=sy[i][0], in_=sy[i][1])
        nc.sync.dma_start(out=wv_all, in_=ap_wv)
        nc.sync.dma_start(out=wo_sb, in_=ap_wo)
        nc.vector.memset(kT[:, n : n + 128], 0.0)
        nc.sync.dma_start(out=kT[:, n : n + M], in_=ap_mkT)
        make_identity(nc, ident[:])
        nc.gpsimd.memset(ones1[:], 1.0)
        for h in range(2):
            nc.vector.memset(v_aug[h][:], 1.0)
            mb = (n_jt - 1) * VA
            nc.vector.memset(v_aug[h][:, mb : mb + VA], 0.0)
            nc.vector.memset(v_aug[h][0:M, mb + 64 : mb + VA], 1.0)
            nc.sync.dma_start(
                out=v_aug[h][0:M, mb : mb + 64],
                in_=ap_mv[:, h * 64 : (h + 1) * 64],
            )

        # ---- minimal pre-attention projections: kT chunk 0 and qT chunk 0
        # only — attention starts as soon as they land; kT chunks 1-3, vT,
        # transposes and remaining qT chunks ride inside the Q0 window as
        # PE filler while the rest of x streams in.
        with tc.tile_pool(name="proj_ps", bufs=4, space="PSUM") as proj_ps:
            for name, w_sb, dst in (("kps0", wk_sb, kT), ("q0ps", wq_sb, qT)):
                ps = proj_ps.tile([128, 512], F32, tag="proj", name=name)
                for c in range(4):
                    nc.tensor.matmul(
                        ps[:],
                        w_sb[c][:],
                        xt[c][:, 0:512],
                        start=(c == 0),
                        stop=(c == 3),
                    )
                nc.scalar.copy(out=dst[:, 0:512], in_=ps[:])

        # ---- attention: i-quarter (512) outer; both heads share one scores
        # PSUM tile (h0 cols 0-511, h1 cols 512-1023) -> one exp call covers
        # both heads; sp double-buffered; av staggered one j-tile behind.
        # The out-projection for quarter q-1 rides inside quarter q's window;
        # its PSUM tiles share the "mix" pool with the av accumulators and
        # deferred q/v projections.
        n_iq = n // 512

        def outproj_quarter(iq, half, mix_pool, stage_pool, tail=False, sp_pool=None):
            ts0 = iq * 4 + (2 if half else 0)
            for t in range(ts0, ts0 + 2):
                if tail and t % 2 == 1:
                    p01 = sp_pool.tile([128, 1024], F32, tag="sp", name="p01")
                    p0 = p01[:, 0:512]
                    p1 = p01[:, 512:1024]
                else:
                    p0 = mix_pool.tile([128, 512], F32, tag="mix", name="p0")
                    p1 = mix_pool.tile([128, 512], F32, tag="mix", name="p1")
                nc.tensor.matmul(
                    p0[:],
                    oT[0:64, t * 128 : (t + 1) * 128],
                    wo_sb[0:64, :],
                    start=True,
                    stop=True,
                )
                nc.tensor.matmul(
                    p1[:],
                    oT[64:128, t * 128 : (t + 1) * 128],
                    wo_sb[64:128, :],
                    start=True,
                    stop=True,
                )
                a1 = stage_pool.tile([128, 512], F32, tag="a1", name="a1")
                if tail:
                    # ACT is idle after the last exp — all h1 scales there; the
                    # combine alternates DVE/Pool so no engine serializes.
                    nc.scalar.activation(
                        out=a1[:],
                        in_=p1[:],
                        func=AF.Copy,
                        scale=rec_col[:, n_it + t : n_it + t + 1],
                    )
                else:
                    nc.vector.tensor_scalar_mul(
                        a1[:], p1[:], rec_col[:, n_it + t : n_it + t + 1]
                    )
                outb = stage_pool.tile([128, 512], BF, tag="outb", name="outb")
                nc.vector.scalar_tensor_tensor(
                    out=outb[:],
                    in0=p0[:],
                    scalar=rec_col[:, t : t + 1],
                    in1=a1[:],
                    op0=ALU.mult,
                    op1=ALU.add,
                )
                if tail:
                    eng = (nc.sync, nc.scalar)[t % 2]
                else:
                    eng = nc.sync if t % 2 == 0 else nc.gpsimd
                eng.dma_start(out=ap_out[t], in_=outb[:])

        def kproj_chunk(k, mix_pool):
            kps = mix_pool.tile([128, 512], F32, tag="mix", name=f"kps{k}")
            for c in range(4):
                nc.tensor.matmul(
                    kps[:],
                    wk_sb[c][:],
                    xt[c][:, k * 512 : (k + 1) * 512],
                    start=(c == 0),
                    stop=(c == 3),
                )
            nc.scalar.copy(out=kT[:, k * 512 : (k + 1) * 512], in_=kps[:])

        with (
            tc.tile_pool(name="s_ps", bufs=2, space="PSUM") as s_ps_pool,
            tc.tile_pool(name="mix_ps", bufs=4, space="PSUM") as mix_ps,
            tc.tile_pool(name="exp_sb", bufs=11) as exp_pool,
            tc.tile_pool(name="small", bufs=4) as small,
            tc.tile_pool(name="ostage", bufs=4) as ostage,
        ):
            def do_rec(riq, dens):
                for h in range(2):
                    dc = mix_ps.tile([128, 4], F32, tag="mix", name="dc")
                    for t in range(4):
                        nc.tensor.transpose(
                            dc[:, t : t + 1],
                            dens[h][0:1, t * 128 : (t + 1) * 128],
                            ones1[:],
                        )
                    nc.vector.reciprocal(
                        out=rec_col[:, h * n_it + riq * 4 : h * n_it + riq * 4 + 4],
                        in_=dc[:],
                    )

            deferred_rec = None
            for iq in range(n_iq):
                i0 = iq * 512
                avs = [
                    mix_ps.tile([VA, 512], F32, tag="mix", name=f"av{h}")
                    for h in range(2)
                ]
                pending = []
                for jt in range(n_jt):
                    sp = s_ps_pool.tile([128, 1024], F32, tag="sp", name="sp")
                    for h in range(2):
                        hp = h * 64
                        nc.tensor.matmul(
                            sp[:, h * 512 : (h + 1) * 512],
                            kT[hp : hp + 64, jt * 128 : (jt + 1) * 128],
                            qT[hp : hp + 64, i0 : i0 + 512],
                            start=True,
                            stop=True,
                        )
                    et = exp_pool.tile([128, 1024], BF, tag="exp", name="et")
                    nc.scalar.activation(out=et[:], in_=sp[:], func=AF.Exp)
                    pending.append((et, jt))
                    if jt == 1 and deferred_rec is not None:
                        do_rec(*deferred_rec)
                        deferred_rec = None
                    if iq == 0 and jt in (2, 4, 6):
                        # kT chunks 1-3 ride just behind the arriving x stream
                        kproj_chunk(jt // 2, mix_ps)
                    if iq == 0 and jt in (7, 9):
                        icp = 0 if jt == 7 else 2
                        vps = [
                            mix_ps.tile([128, 512], F32, tag="mix", name="vps")
                            for _ in range(2)
                        ]
                        for c in range(4):
                            for k in range(2):
                                nc.tensor.matmul(
                                    vps[k][:],
                                    wv_sb[c][:],
                                    xt[c][:, (icp + k) * 512 : (icp + k + 1) * 512],
                                    start=(c == 0),
                                    stop=(c == 3),
                                )
                        for k in range(2):
                            nc.vector.tensor_copy(
                                out=vT[:, (icp + k) * 512 : (icp + k + 1) * 512],
                                in_=vps[k][:],
                            )
                    if iq == 0 and jt in (8, 10):
                        t0 = 0 if jt == 8 else 8
                        for tjt in range(t0, t0 + 8):
                            pt = mix_ps.tile([128, 128], BF, tag="mix", name="tr")
                            nc.tensor.transpose(
                                pt[:], vT[:, tjt * 128 : (tjt + 1) * 128], ident[:]
                            )
                            for h in range(2):
                                nc.vector.tensor_copy(
                                    out=v_aug[h][:, tjt * VA : tjt * VA + 64],
                                    in_=pt[:, h * 64 : (h + 1) * 64],
                                )
                    if iq == 0:
                        n_pop = 0
                        if jt >= 9:
                            n_pop = 2 if len(pending) > 6 else 1
                    else:
                        n_pop = 1 if len(pending) > 2 else 0
                    for _ in range(n_pop):
                        pet, pjt = pending.pop(0)
                        for h in range(2):
                            nc.tensor.matmul(
                                avs[h][:],
                                v_aug[h][:, pjt * VA : (pjt + 1) * VA],
                                pet[:, h * 512 : (h + 1) * 512],
                                start=(pjt == 0),
                                stop=False,
                            )
                    if iq == 0 and jt in (11, 13, 15):
                        # deferred qT chunk, one per insertion point
                        ic = (jt - 11) // 2 + 1
                        qp = mix_ps.tile([128, 512], F32, tag="mix", name="qdef")
                        for c in range(4):
                            nc.tensor.matmul(
                                qp[:],
                                wq_sb[c][:],
                                xt[c][:, ic * 512 : (ic + 1) * 512],
                                start=(c == 0),
                                stop=(c == 3),
                            )
                        nc.vector.tensor_copy(
                            out=qT[:, ic * 512 : (ic + 1) * 512], in_=qp[:]
                        )
                    if jt in (4, 8) and iq >= 1:
                        # out-projection for the previous quarter rides here
                        # (2 tiles per insertion), round-trip long completed
                        outproj_quarter(iq - 1, jt == 8, mix_ps, ostage)
                while pending:
                    pet, pjt = pending.pop(0)
                    for h in range(2):
                        nc.tensor.matmul(
                            avs[h][:],
                            v_aug[h][:, pjt * VA : (pjt + 1) * VA],
                            pet[:, h * 512 : (h + 1) * 512],
                            start=(pjt == 0),
                            stop=(pjt == n_jt - 1),
                        )
                # epilogue for this i-quarter: pull the denominator rows and
                # oT out of PSUM; the rec computation (PE transposes + recip)
                # is deferred into the next quarter's window so the PE never
                # idles at the boundary.  DVE does the copies mid-run; ACT
                # (idle after the last exp) takes them on the final quarter.
                dens = []
                for h in range(2):
                    den = small.tile([1, 512], F32, tag="den", name="den")
                    if iq == n_iq - 1 and h == 0:
                        nc.scalar.copy(out=den[:], in_=avs[h][64:65, :])
                    else:
                        nc.vector.tensor_copy(out=den[:], in_=avs[h][64:65, :])
                    dens.append(den)
                for h in range(2):
                    hp = h * 64
                    nc.vector.tensor_copy(
                        out=oT[hp : hp + 64, i0 : i0 + 512], in_=avs[h][0:64, :]
                    )
                deferred_rec = (iq, dens)
                if iq == n_iq - 1:
                    do_rec(*deferred_rec)
            # final quarter's out-projection (ACT helps with the scaling)
            outproj_quarter(n_iq - 1, False, mix_ps, ostage, tail=True, sp_pool=s_ps_pool)
            outproj_quarter(n_iq - 1, True, mix_ps, ostage, tail=True, sp_pool=s_ps_pool)


def _build(n=NSEQ):
    if n in _CACHE:
        return _CACHE[n]
    nc = bacc.Bacc("TRN2", debug=False, num_devices=N_CORES)
    with tile.TileContext(nc) as tc:
        _emit(nc, tc, n)
    nc.compile()
    _CACHE[n] = nc
    return nc


def _prep_in_maps(x, wq, wkv, wo, m_k, m_v, n):
    x = np.asarray(x, np.float32)
    wq = np.asarray(wq, np.float32)
    wkv = np.asarray(wkv, np.float32)
    wo = np.asarray(wo, np.float32)
    m_k = np.asarray(m_k, np.float32)
    m_v = np.asarray(m_v, np.float32)

    wk = wkv[:, :INNER]
    wv = wkv[:, INNER:]
    # memory tokens: flat reshape (M, INNER) -> (H, M, DH), exactly as reference
    mk_heads = m_k.reshape(M * INNER).reshape(H, M, DH)  # * SQRT_DH * SCALE == 1.0
    mv_heads = m_v.reshape(M * INNER).reshape(H, M, DH) * SQRT_M

    in_maps = []
    for cid in range(N_CORES):
        b = cid // 4
        h0 = 2 * (cid % 4)
        sl = slice(h0 * DH, (h0 + 2) * DH)
        in_maps.append(
            {
                "xt": np.ascontiguousarray(x[b].T)
                .reshape(4, 128, n)
                .astype(ml_dtypes.bfloat16),
                "wq_s": np.ascontiguousarray(
                    wq[:, sl].reshape(4, 128, 128).transpose(1, 0, 2).reshape(128, 512)
                ).astype(ml_dtypes.bfloat16),
                "wk_s": np.ascontiguousarray(
                    (wk[:, sl] * SCALE)
                    .reshape(4, 128, 128)
                    .transpose(1, 0, 2)
                    .reshape(128, 512)
                ).astype(ml_dtypes.bfloat16),
                "wv_s": np.ascontiguousarray(
                    wv[:, sl].reshape(4, 128, 128).transpose(1, 0, 2).reshape(128, 512)
                ).astype(ml_dtypes.bfloat16),
                "wo_s": np.ascontiguousarray(wo[sl, :]).astype(ml_dtypes.bfloat16),
                "mkT_s": np.ascontiguousarray(
                    np.concatenate([mk_heads[h0].T, mk_heads[h0 + 1].T], axis=0)
                ).astype(ml_dtypes.bfloat16),
                "mv_s": np.ascontiguousarray(
                    np.concatenate([mv_heads[h0], mv_heads[h0 + 1]], axis=1)
                ).astype(ml_dtypes.bfloat16),
            }
        )
    return in_maps


def _gather(results, bo, n):
    bo = np.asarray(bo, np.float32)
    out = np.zeros((B, n, DIM), np.float32)
    for cid in range(N_CORES):
        out[cid // 4] += results[cid]["out"].reshape(n, DIM).astype(np.float32)
    out += bo
    return out


def run(x, wq, wkv, wo, bo, m_k, m_v, trace=False, n=NSEQ):
    nc = _build(n)
    in_maps = _prep_in_maps(x, wq, wkv, wo, m_k, m_v, n)
    res = bass_utils.run_bass_kernel_spmd(
        nc, in_maps, core_ids=list(range(N_CORES)), trace=trace
    )
    return _gather(res.results, bo, n), res


def kernel(x, wq, wkv, wo, bo, m_k, m_v):
    out, _ = run(x, wq, wkv, wo, bo, m_k, m_v)
    return out
